# revision 13
# baseline (speedup 1.0000x reference)
"""Trainium2 Bass kernel for nn_Attention_40759239639448.

Full-input contract: kernel(**inputs) takes the unsharded inputs and returns
the full [B, T, C] output. Internally shards across 8 NeuronCores:
data-parallel over nothing, tensor-parallel over heads (2 heads per core,
both batch elements on every core); each core computes a partial
out-projection over its 128 head-channels and the host sums the 8 partials.

Per-core pipeline (all matmuls in float32r: 1 cyc/row at fp32-ish precision):
  phase 1: qkv projection in transposed [d, t] layout + RoPE + RMSNorm
           (rope weight-norm folded into host trig tables; rsqrt applied via
           PE broadcast), v transposed back to [t, d] via PE transpose.
  phase 2: causal attention per (batch, 512-row q-tile): S^T = k^T q in
           [k, q] layout (two heads packed as PE row-tiles), exp on ACT,
           staircase causal mask on DVE, P^T @ V_aug accumulation where
           V_aug carries a ones column producing the softmax denominator.
  phase 2.5 per batch: denominator reciprocal (DMA-reshaped to [64, 64] so
           the exact DVE reciprocal runs wide), PE broadcast, normalize into
           yT, then the out-projection and DMA of the fp32 partial result.
"""
import sys

sys.path.insert(0, "/opt/trn_rl_repo")

import numpy as np

B, T, C, H = 2, 2048, 1024, 16
D = C // H            # 64
NC = 8                # cores
HPC = H // NC         # 2 heads per core
TT = B * T            # 4096 flattened tokens
EPS = 1e-6
ROPE_BASE = 10000.0
NCH = 8               # t-chunks of 512
CH = 512              # chunk width (tokens)
KT = 128              # k-tile rows
QT = 512              # q-tile width
NKT = T // KT         # 16 k-tiles per batch
NQT = T // QT         # 4 q-tiles per batch

_cache = {}


def _build():
    import concourse.bacc as bacc
    import concourse.mybir as mybir
    import concourse.tile as tile

    f32 = mybir.dt.float32
    f32r = mybir.dt.float32r
    AF = mybir.ActivationFunctionType

    nc = bacc.Bacc(None, target_bir_lowering=False)

    # ---- DRAM I/O ----
    xT_d = nc.dram_tensor("xT", [C, TT], f32r, kind="ExternalInput")
    wqT_d = nc.dram_tensor("wqT", [C, 128], f32r, kind="ExternalInput")
    wkT_d = nc.dram_tensor("wkT", [C, 128], f32r, kind="ExternalInput")
    wvT_d = nc.dram_tensor("wvT", [C, 128], f32r, kind="ExternalInput")
    woT_d = nc.dram_tensor("woT", [128, C], f32r, kind="ExternalInput")
    cosq_d = nc.dram_tensor("cosq", [128, T], f32, kind="ExternalInput")
    sinq_d = nc.dram_tensor("sinq", [128, T], f32, kind="ExternalInput")
    cosk_d = nc.dram_tensor("cosk", [128, T], f32, kind="ExternalInput")
    sink_d = nc.dram_tensor("sink", [128, T], f32, kind="ExternalInput")
    mask2_d = nc.dram_tensor("mask2", [128, 4096], f32r, kind="ExternalInput")
    e2_d = nc.dram_tensor("e2", [2, 128], f32r, kind="ExternalInput")
    bd_d = nc.dram_tensor("bd", [128, 2], f32r, kind="ExternalInput")
    onesr_d = nc.dram_tensor("onesr", [65, 64], f32, kind="ExternalInput")
    ones32_d = nc.dram_tensor("ones32", [128, 32], f32r, kind="ExternalInput")
    ident_d = nc.dram_tensor("ident", [128, 128], f32r, kind="ExternalInput")
    epsb_d = nc.dram_tensor("epsb", [128, 1], f32, kind="ExternalInput")
    out_d = nc.dram_tensor("out", [TT, C], f32, kind="ExternalOutput")

    with tile.TileContext(nc) as tc:
        with tc.tile_pool(name="persist", bufs=1) as pp:
            qT = pp.tile([128, TT], f32r, tag="qT")
            kT = pp.tile([128, TT], f32r, tag="kT")
            yT = pp.tile([128, TT], f32r, tag="yT")
            vaug0 = pp.tile([128, B * NKT * 65], f32r, tag="vaug0")
            vaug1 = pp.tile([128, B * NKT * 65], f32r, tag="vaug1")
            wo_sb = pp.tile([128, C], f32r, tag="wo")
            e2_sb = pp.tile([2, 128], f32r, tag="e2")
            bd_sb = pp.tile([128, 2], f32r, tag="bd")
            onesr = pp.tile([65, 64], f32, tag="onesr")
            ones32 = pp.tile([128, 32], f32r, tag="ones32")
            ident = pp.tile([128, 128], f32r, tag="ident")
            epsb = pp.tile([128, 1], f32, tag="epsb")

            nc.sync.dma_start(wo_sb[:], woT_d[:])
            nc.sync.dma_start(e2_sb[:], e2_d[:])
            nc.sync.dma_start(bd_sb[:], bd_d[:])
            nc.sync.dma_start(onesr[:], onesr_d[:])
            nc.sync.dma_start(ones32[:], ones32_d[:])
            nc.sync.dma_start(ident[:], ident_d[:])
            nc.sync.dma_start(epsb[:], epsb_d[:])
            # ones columns of V_aug (DVE-produced so f32r matmul consumers
            # see a compute-engine producer)
            for va in (vaug0, vaug1):
                nc.vector.tensor_copy(
                    va[:].rearrange("p (i f) -> p i f", f=65)[:, :, 64],
                    ones32[:, 0 : B * NKT],
                )

            # ================= phase 1: QKV + RoPE + RMSNorm =================
            with (
                tc.tile_pool(name="p1sb", bufs=1) as p1,
                tc.tile_pool(name="xp", bufs=11) as xp,
                tc.tile_pool(name="scr", bufs=3) as scr,
                tc.tile_pool(name="ps1", bufs=3, space="PSUM") as qkvp,
                tc.tile_pool(name="ps1b", bufs=2, space="PSUM") as msp,
                tc.tile_pool(name="ps1c", bufs=1, space="PSUM") as rsbcp,
                tc.tile_pool(name="ps1d", bufs=2, space="PSUM") as scrp,
            ):
                wq_sb = p1.tile([128, C], f32r, tag="wq")
                wk_sb = p1.tile([128, C], f32r, tag="wk")
                wv_sb = p1.tile([128, C], f32r, tag="wv")
                vT = p1.tile([128, TT], f32r, tag="vT")
                cosq = p1.tile([128, T], f32, tag="cosq")
                sinq = p1.tile([128, T], f32, tag="sinq")
                cosk = p1.tile([128, T], f32, tag="cosk")
                sink = p1.tile([128, T], f32, tag="sink")
                for cc in range(8):
                    nc.sync.dma_start(
                        wq_sb[:, 128 * cc : 128 * cc + 128],
                        wqT_d[128 * cc : 128 * cc + 128, :],
                    )
                    nc.sync.dma_start(
                        wk_sb[:, 128 * cc : 128 * cc + 128],
                        wkT_d[128 * cc : 128 * cc + 128, :],
                    )
                    nc.sync.dma_start(
                        wv_sb[:, 128 * cc : 128 * cc + 128],
                        wvT_d[128 * cc : 128 * cc + 128, :],
                    )
                nc.sync.dma_start(cosq[:], cosq_d[:])
                nc.sync.dma_start(sinq[:], sinq_d[:])
                nc.sync.dma_start(cosk[:], cosk_d[:])
                nc.sync.dma_start(sink[:], sink_d[:])

                for ci in range(NCH):
                    t0 = ci * CH          # global token col
                    tt0 = t0 % T          # within-batch col (trig tables)
                    b = t0 // T
                    xts = []
                    for cc in range(8):
                        xt = xp.tile([128, CH], f32r, tag="x")
                        nc.sync.dma_start(
                            xt[:], xT_d[128 * cc : 128 * cc + 128, t0 : t0 + CH]
                        )
                        xts.append(xt)
                    q_ps = qkvp.tile([128, CH], f32, tag="qkv")
                    k_ps = qkvp.tile([128, CH], f32, tag="qkv")
                    for cc in range(8):
                        nc.tensor.matmul(
                            q_ps[:], wq_sb[:, 128 * cc : 128 * cc + 128],
                            xts[cc][:], start=(cc == 0), stop=(cc == 7),
                        )
                    for cc in range(8):
                        nc.tensor.matmul(
                            k_ps[:], wk_sb[:, 128 * cc : 128 * cc + 128],
                            xts[cc][:], start=(cc == 0), stop=(cc == 7),
                        )

                    # rms statistics (rope-invariant): sq -> blockdiag-sum
                    sq_q = scr.tile([128, CH], f32r, tag="sq")
                    sq_k = scr.tile([128, CH], f32r, tag="sq")
                    nc.scalar.activation(sq_q[:], q_ps[:], AF.Square)
                    nc.scalar.activation(sq_k[:], k_ps[:], AF.Square)
                    ms_q = msp.tile([2, CH], f32, tag="ms")
                    ms_k = msp.tile([2, CH], f32, tag="ms")
                    nc.tensor.matmul(ms_q[:], bd_sb[:], sq_q[:], start=True, stop=True)
                    nc.tensor.matmul(ms_k[:], bd_sb[:], sq_k[:], start=True, stop=True)
                    def rs_chain(ms_ps, name):
                        ms_sb = scr.tile([2, CH], f32, tag="ms_" + name)
                        nc.scalar.copy(ms_sb[:], ms_ps[:])
                        m2 = scr.tile([64, 16], f32, tag="m2_" + name)
                        nc.sync.dma_start(m2[:], ms_sb[:])
                        std2 = scr.tile([64, 16], f32, tag="st_" + name)
                        nc.scalar.activation(
                            std2[:], m2[:], AF.Sqrt, scale=1.0 / D,
                            bias=epsb[0:64, :],
                        )
                        r2 = scr.tile([64, 16], f32r, tag="r2_" + name)
                        with nc.allow_low_precision(reason="rsqrt to f32r"):
                            nc.vector.reciprocal(r2[:], std2[:])
                        rs4 = scr.tile([2, CH], f32r, tag="rs_" + name)
                        nc.sync.dma_start(rs4[:], r2[:])
                        return rs4

                    rs_q4 = rs_chain(ms_q, "q")
                    rs_k4 = rs_chain(ms_k, "k")
                    rsbc_q = rsbcp.tile([128, CH], f32, tag="rsbc")
                    nc.tensor.matmul(
                        rsbc_q[:], e2_sb[:], rs_q4[:], start=True, stop=True
                    )

                    # rope: tc = x*cos (sbuf), ts = x*sin (psum, for the
                    # partition-shifted combine), o = tc -+ ts_shifted
                    tcq = scr.tile([128, 2 * CH], f32, tag="tc")
                    nc.vector.tensor_mul(
                        tcq[:, 0:CH], q_ps[:], cosq[:, tt0 : tt0 + CH]
                    )
                    nc.vector.tensor_mul(
                        tcq[:, CH : 2 * CH], k_ps[:], cosk[:, tt0 : tt0 + CH]
                    )
                    ts_q = scrp.tile([128, CH], f32, tag="tsp")
                    ts_k = scrp.tile([128, CH], f32, tag="tsp")
                    nc.vector.tensor_mul(ts_q[:], q_ps[:], sinq[:, tt0 : tt0 + CH])
                    nc.vector.tensor_mul(ts_k[:], k_ps[:], sink[:, tt0 : tt0 + CH])
                    o_ = scr.tile([128, 2 * CH], f32, tag="o_")

                    def addsub(lo, hi, sub_first):
                        # o[lo:lo+32] = tc[lo:lo+32] - ts[hi:hi+32]  (x1 rows)
                        # o[hi:hi+32] = tc[hi:hi+32] + ts[lo:lo+32]  (x2 rows)
                        for (ts_t, off) in ((ts_q, 0), (ts_k, CH)):
                            sl = slice(off, off + CH)
                            nc.vector.tensor_sub(
                                o_[lo : lo + 32, sl],
                                tcq[lo : lo + 32, sl],
                                ts_t[hi : hi + 32, :],
                            )
                            nc.vector.tensor_add(
                                o_[hi : hi + 32, sl],
                                tcq[hi : hi + 32, sl],
                                ts_t[lo : lo + 32, :],
                            )

                    addsub(0, 32, True)
                    addsub(64, 96, True)

                    nc.vector.tensor_mul(
                        qT[:, t0 : t0 + CH], o_[:, 0:CH], rsbc_q[:]
                    )
                    rsbc_k = rsbcp.tile([128, CH], f32, tag="rsbc")
                    nc.tensor.matmul(
                        rsbc_k[:], e2_sb[:], rs_k4[:], start=True, stop=True
                    )
                    nc.vector.tensor_mul(
                        kT[:, t0 : t0 + CH], o_[:, CH : 2 * CH], rsbc_k[:]
                    )

                    # v: plain projection then transpose each 128-token tile
                    v_ps = qkvp.tile([128, CH], f32, tag="qkv")
                    for cc in range(8):
                        nc.tensor.matmul(
                            v_ps[:], wv_sb[:, 128 * cc : 128 * cc + 128],
                            xts[cc][:], start=(cc == 0), stop=(cc == 7),
                        )
                    nc.scalar.copy(vT[:, t0 : t0 + CH], v_ps[:])
                    for jj in range(4):
                        kti = (tt0 // KT) + jj          # k-tile within batch
                        col = t0 + jj * KT
                        vt_ps = scrp.tile([128, 128], f32r, tag="tsp")
                        nc.tensor.transpose(
                            vt_ps[:], vT[:, col : col + 128], ident[:]
                        )
                        base = (b * NKT + kti) * 65
                        nc.scalar.copy(vaug0[:, base : base + 64], vt_ps[:, 0:64])
                        nc.scalar.copy(vaug1[:, base : base + 64], vt_ps[:, 64:128])

            # ================= phase 2: attention =================
            with (
                tc.tile_pool(name="p2sb", bufs=1) as p2,
                tc.tile_pool(name="pp2", bufs=8) as ppool,
                tc.tile_pool(name="op2", bufs=8) as opool,
                tc.tile_pool(name="ps2", bufs=3, space="PSUM") as spsp,
                tc.tile_pool(name="ps2y", bufs=1, space="PSUM") as yp,
            ):
                yraw = p2.tile([65, 4096], f32, tag="yraw")
                mask2 = p2.tile([128, 4096], f32r, tag="mask2")
                nc.sync.dma_start(mask2[:], mask2_d[:])
                den2 = p2.tile([64, 64], f32, tag="den2")
                rcp2 = p2.tile([64, 64], f32, tag="rcp2")

                for b in range(B):
                    bt = b * T
                    for qi in range(NQT):
                        q0 = bt + qi * QT
                        nk = 4 * qi + 4
                        y_ps = yp.tile([65, 2 * QT], f32, tag="y")
                        for ki in range(nk):
                            k0 = bt + ki * KT
                            s_ps = spsp.tile([128, 2 * QT], f32, tag="sps")
                            nc.tensor.matmul(
                                s_ps[:, 0:QT],
                                kT[0:64, k0 : k0 + KT],
                                qT[0:64, q0 : q0 + QT],
                                start=True, stop=True, tile_position=(0, 0),
                            )
                            nc.tensor.matmul(
                                s_ps[:, QT : 2 * QT],
                                kT[64:128, k0 : k0 + KT],
                                qT[64:128, q0 : q0 + QT],
                                start=True, stop=True, tile_position=(64, 0),
                            )
                            p_sb = ppool.tile([128, 2 * QT], f32r, tag="p")
                            nc.scalar.activation(
                                p_sb[:], s_ps[:], AF.Exp, scale=0.125
                            )
                            mi = ki - 4 * qi
                            if mi >= 0:
                                # staircase mask on the leading m+128 columns
                                msz = min(mi * KT + KT, QT)
                                pv = p_sb[:].rearrange(
                                    "p (h q) -> p h q", h=2
                                )[:, :, 0:msz]
                                mv = mask2[
                                    :, 1024 * mi : 1024 * mi + 1024
                                ].rearrange("p (h q) -> p h q", h=2)[:, :, 0:msz]
                                eng = nc.gpsimd if mi >= 2 else nc.vector
                                eng.tensor_mul(pv, pv, mv)
                            base = (b * NKT + ki) * 65
                            nc.tensor.matmul(
                                y_ps[:, 0:QT],
                                vaug0[:, base : base + 65],
                                p_sb[:, 0:QT],
                                start=(ki == 0), stop=(ki == nk - 1),
                            )
                            nc.tensor.matmul(
                                y_ps[:, QT : 2 * QT],
                                vaug1[:, base : base + 65],
                                p_sb[:, QT : 2 * QT],
                                start=(ki == 0), stop=(ki == nk - 1),
                            )
                        # stash raw y + den; cols h*2048 + qi*512
                        nc.vector.tensor_copy(
                            yraw[:, 0 * T + qi * QT : 0 * T + qi * QT + QT],
                            y_ps[:, 0:QT],
                        )
                        nc.vector.tensor_copy(
                            yraw[:, 1 * T + qi * QT : 1 * T + qi * QT + QT],
                            y_ps[:, QT : 2 * QT],
                        )

                    # ---- phase 2.5: normalize + out-projection for batch b
                    nc.sync.dma_start(
                        den2[:],
                        yraw[64:65, :].rearrange("o (p f) -> o p f", p=64),
                    )
                    nc.vector.reciprocal(rcp2[:], den2[:])
                    nc.sync.dma_start(
                        yraw[64:65, :].rearrange("o (p f) -> o p f", p=64),
                        rcp2[:],
                    )
                    for h in range(2):
                        for half in range(2):
                            c0 = h * T + half * 1024
                            db_ps = spsp.tile([64, 1024], f32, tag="sps")
                            nc.tensor.matmul(
                                db_ps[:, 0:512],
                                onesr[64:65, :],
                                yraw[64:65, c0 : c0 + 512],
                                start=True, stop=True,
                            )
                            nc.tensor.matmul(
                                db_ps[:, 512:1024],
                                onesr[64:65, :],
                                yraw[64:65, c0 + 512 : c0 + 1024],
                                start=True, stop=True,
                            )
                            nc.vector.tensor_mul(
                                yT[64 * h : 64 * h + 64,
                                   bt + half * 1024 : bt + half * 1024 + 1024],
                                yraw[0:64, c0 : c0 + 1024],
                                db_ps[:],
                            )
                    for tt in range(16):
                        tg = bt + tt * 128
                        o_ps = spsp.tile([128, C], f32, tag="sps")
                        nc.tensor.matmul(
                            o_ps[:, 0:512], yT[:, tg : tg + 128],
                            wo_sb[:, 0:512], start=True, stop=True,
                        )
                        nc.tensor.matmul(
                            o_ps[:, 512:1024], yT[:, tg : tg + 128],
                            wo_sb[:, 512:1024], start=True, stop=True,
                        )
                        o_sb = opool.tile([128, C], f32, tag="osb")
                        if tt % 4 == 3:
                            nc.scalar.copy(o_sb[:], o_ps[:])
                        else:
                            nc.vector.tensor_copy(o_sb[:], o_ps[:])
                        nc.sync.dma_start(out_d[tg : tg + 128, :], o_sb[:])

    nc.finalize()
    return nc


def _host_prep(x, w_qkv, w_out, q_norm_w, k_norm_w):
    xT = np.ascontiguousarray(x.reshape(TT, C).T).astype(np.float32)

    j = np.arange(32, dtype=np.float64)
    inv = ROPE_BASE ** (-j / 32.0)
    tt = np.arange(T, dtype=np.float64)
    ang = tt[:, None] * inv[None, :]          # [T, 32]
    cos_t = np.cos(ang)                        # [T, 32]
    sin_t = np.sin(ang)

    def trig_tables(w):
        w = np.asarray(w, dtype=np.float64)
        cosr = np.empty((128, T), np.float32)
        sinr = np.empty((128, T), np.float32)
        for p in range(128):
            jj = p % 32
            r = p % 64
            cosr[p] = (cos_t[:, jj] * w[r]).astype(np.float32)
            sinr[p] = (sin_t[:, jj] * w[(r + 32) % 64]).astype(np.float32)
        return cosr, sinr

    cosq, sinq = trig_tables(q_norm_w)
    cosk, sink = trig_tables(k_norm_w)

    mask2 = np.zeros((128, 4096), np.float32)
    kp = np.arange(128)[:, None]
    qq = np.arange(512)[None, :]
    for mi, m in enumerate((0, 128, 256, 384)):
        blk = (kp + m <= qq).astype(np.float32)
        mask2[:, 1024 * mi : 1024 * mi + 512] = blk
        mask2[:, 1024 * mi + 512 : 1024 * mi + 1024] = blk

    e2 = np.zeros((2, 128), np.float32)
    e2[0, 0:64] = 1.0
    e2[1, 64:128] = 1.0
    bd = np.zeros((128, 2), np.float32)
    bd[0:64, 0] = 1.0
    bd[64:128, 1] = 1.0
    onesr = np.ones((65, 64), np.float32)
    ones32 = np.ones((128, 32), np.float32)
    ident = np.eye(128, dtype=np.float32)

    shared = {
        "xT": xT, "cosq": cosq, "sinq": sinq, "cosk": cosk, "sink": sink,
        "mask2": mask2, "e2": e2, "bd": bd, "onesr": onesr,
        "ones32": ones32, "ident": ident,
        "epsb": np.full((128, 1), EPS, np.float32),
    }

    in_maps = []
    for c in range(NC):
        rows = np.r_[2 * c * 64 : (2 * c + 1) * 64,
                     (2 * c + 1) * 64 : (2 * c + 2) * 64]
        wq = w_qkv[rows, :]
        wk = w_qkv[C + rows, :]
        wv = w_qkv[2 * C + rows, :]
        m = dict(shared)
        m["wqT"] = np.ascontiguousarray(wq.T).astype(np.float32)
        m["wkT"] = np.ascontiguousarray(wk.T).astype(np.float32)
        m["wvT"] = np.ascontiguousarray(wv.T).astype(np.float32)
        m["woT"] = np.ascontiguousarray(w_out[:, rows].T).astype(np.float32)
        in_maps.append(m)
    return in_maps


def kernel(x, w_qkv, w_out, q_norm_w, k_norm_w, _trace=False):
    from concourse.bass_utils import run_bass_kernel_spmd

    if "nc" not in _cache:
        _cache["nc"] = _build()
    nc = _cache["nc"]

    x = np.asarray(x, dtype=np.float32)
    w_qkv = np.asarray(w_qkv, dtype=np.float32)
    w_out = np.asarray(w_out, dtype=np.float32)
    q_norm_w = np.asarray(q_norm_w, dtype=np.float32)
    k_norm_w = np.asarray(k_norm_w, dtype=np.float32)

    in_maps = _host_prep(x, w_qkv, w_out, q_norm_w, k_norm_w)
    res = run_bass_kernel_spmd(
        nc, in_maps, list(range(NC)), trace=_trace,
    )
    _cache["last_result"] = res
    parts = np.stack([r["out"] for r in res.results], axis=0)
    out = parts.sum(axis=0, dtype=np.float64).astype(np.float32)
    return out.reshape(B, T, C)


# revision 14
# speedup vs baseline: 1.8189x; 1.8189x over previous
"""Trainium2 Bass kernel for nn_Attention_40759239639448.

Full-input contract: kernel(**inputs) takes the unsharded inputs and returns
the full [B, T, C] output. Internally shards across 8 NeuronCores:
data-parallel over nothing, tensor-parallel over heads (2 heads per core,
both batch elements on every core); each core computes a partial
out-projection over its 128 head-channels and the host sums the 8 partials.

Per-core pipeline (all matmuls in float32r: 1 cyc/row at fp32-ish precision):
  phase 1: qkv projection in transposed [d, t] layout + RoPE + RMSNorm
           (rope weight-norm folded into host trig tables; rsqrt applied via
           PE broadcast), v transposed back to [t, d] via PE transpose.
  phase 2: causal attention per (batch, 512-row q-tile): S^T = k^T q in
           [k, q] layout (two heads packed as PE row-tiles), exp on ACT,
           staircase causal mask on DVE, P^T @ V_aug accumulation where
           V_aug carries a ones column producing the softmax denominator.
  phase 2.5 per batch: denominator reciprocal (DMA-reshaped to [64, 64] so
           the exact DVE reciprocal runs wide), PE broadcast, normalize into
           yT, then the out-projection and DMA of the fp32 partial result.
"""
import sys

sys.path.insert(0, "/opt/trn_rl_repo")

import numpy as np

B, T, C, H = 2, 2048, 1024, 16
D = C // H            # 64
NC = 8                # cores
HPC = H // NC         # 2 heads per core
TT = B * T            # 4096 flattened tokens
EPS = 1e-6
ROPE_BASE = 10000.0
NCH = 8               # t-chunks of 512
CH = 512              # chunk width (tokens)
KT = 128              # k-tile rows
QT = 512              # q-tile width
NKT = T // KT         # 16 k-tiles per batch
NQT = T // QT         # 4 q-tiles per batch

_cache = {}


def _build():
    import concourse.bacc as bacc
    import concourse.mybir as mybir
    import concourse.tile as tile

    f32 = mybir.dt.float32
    f32r = mybir.dt.float32r
    AF = mybir.ActivationFunctionType

    nc = bacc.Bacc(None, target_bir_lowering=False)

    # ---- DRAM I/O ----
    xT_d = nc.dram_tensor("xT", [C, TT], f32r, kind="ExternalInput")
    wqT_d = nc.dram_tensor("wqT", [C, 128], f32r, kind="ExternalInput")
    wkT_d = nc.dram_tensor("wkT", [C, 128], f32r, kind="ExternalInput")
    wvT_d = nc.dram_tensor("wvT", [C, 128], f32r, kind="ExternalInput")
    woT_d = nc.dram_tensor("woT", [128, C], f32r, kind="ExternalInput")
    cosq_d = nc.dram_tensor("cosq", [128, T], f32, kind="ExternalInput")
    sinq_d = nc.dram_tensor("sinq", [128, T], f32, kind="ExternalInput")
    cosk_d = nc.dram_tensor("cosk", [128, T], f32, kind="ExternalInput")
    sink_d = nc.dram_tensor("sink", [128, T], f32, kind="ExternalInput")
    mask2_d = nc.dram_tensor("mask2", [128, 4096], f32r, kind="ExternalInput")
    e2_d = nc.dram_tensor("e2", [2, 128], f32r, kind="ExternalInput")
    bd_d = nc.dram_tensor("bd", [128, 2], f32r, kind="ExternalInput")
    onesr_d = nc.dram_tensor("onesr", [65, 64], f32, kind="ExternalInput")
    ones32_d = nc.dram_tensor("ones32", [128, 32], f32r, kind="ExternalInput")
    ident_d = nc.dram_tensor("ident", [128, 128], f32r, kind="ExternalInput")
    epsb_d = nc.dram_tensor("epsb", [128, 1], f32, kind="ExternalInput")
    out_d = nc.dram_tensor("out", [TT, C], f32, kind="ExternalOutput")

    with tile.TileContext(nc) as tc:
        with tc.tile_pool(name="persist", bufs=1) as pp:
            qT = pp.tile([128, TT], f32r, tag="qT")
            kT = pp.tile([128, TT], f32r, tag="kT")
            yT = pp.tile([128, TT], f32r, tag="yT")
            vaug0 = pp.tile([128, B * NKT * 65], f32r, tag="vaug0")
            vaug1 = pp.tile([128, B * NKT * 65], f32r, tag="vaug1")
            wo_sb = pp.tile([128, C], f32r, tag="wo")
            e2_sb = pp.tile([2, 128], f32r, tag="e2")
            bd_sb = pp.tile([128, 2], f32r, tag="bd")
            onesr = pp.tile([65, 64], f32, tag="onesr")
            ones32 = pp.tile([128, 32], f32r, tag="ones32")
            ident = pp.tile([128, 128], f32r, tag="ident")
            epsb = pp.tile([128, 1], f32, tag="epsb")

            nc.sync.dma_start(wo_sb[:], woT_d[:])
            nc.sync.dma_start(e2_sb[:], e2_d[:])
            nc.sync.dma_start(bd_sb[:], bd_d[:])
            nc.sync.dma_start(onesr[:], onesr_d[:])
            nc.sync.dma_start(ones32[:], ones32_d[:])
            nc.sync.dma_start(ident[:], ident_d[:])
            nc.sync.dma_start(epsb[:], epsb_d[:])
            # ones columns of V_aug (DVE-produced so f32r matmul consumers
            # see a compute-engine producer)
            for va in (vaug0, vaug1):
                nc.vector.tensor_copy(
                    va[:].rearrange("p (i f) -> p i f", f=65)[:, :, 64],
                    ones32[:, 0 : B * NKT],
                )

            # ================= phase 1: QKV + RoPE + RMSNorm =================
            with (
                tc.tile_pool(name="p1sb", bufs=1) as p1,
                tc.tile_pool(name="xp", bufs=11) as xp,
                tc.tile_pool(name="scr", bufs=3) as scr,
                tc.tile_pool(name="ps1", bufs=3, space="PSUM") as qkvp,
                tc.tile_pool(name="ps1b", bufs=2, space="PSUM") as msp,
                tc.tile_pool(name="ps1c", bufs=1, space="PSUM") as rsbcp,
                tc.tile_pool(name="ps1d", bufs=2, space="PSUM") as scrp,
            ):
                wq_sb = p1.tile([128, C], f32r, tag="wq")
                wk_sb = p1.tile([128, C], f32r, tag="wk")
                wv_sb = p1.tile([128, C], f32r, tag="wv")
                vT = p1.tile([128, TT], f32r, tag="vT")
                cosq = p1.tile([128, T], f32, tag="cosq")
                sinq = p1.tile([128, T], f32, tag="sinq")
                cosk = p1.tile([128, T], f32, tag="cosk")
                sink = p1.tile([128, T], f32, tag="sink")
                for cc in range(8):
                    nc.sync.dma_start(
                        wq_sb[:, 128 * cc : 128 * cc + 128],
                        wqT_d[128 * cc : 128 * cc + 128, :],
                    )
                    nc.sync.dma_start(
                        wk_sb[:, 128 * cc : 128 * cc + 128],
                        wkT_d[128 * cc : 128 * cc + 128, :],
                    )
                    nc.sync.dma_start(
                        wv_sb[:, 128 * cc : 128 * cc + 128],
                        wvT_d[128 * cc : 128 * cc + 128, :],
                    )
                nc.sync.dma_start(cosq[:], cosq_d[:])
                nc.sync.dma_start(sinq[:], sinq_d[:])
                nc.sync.dma_start(cosk[:], cosk_d[:])
                nc.sync.dma_start(sink[:], sink_d[:])

                for ci in range(NCH):
                    t0 = ci * CH          # global token col
                    tt0 = t0 % T          # within-batch col (trig tables)
                    b = t0 // T
                    xts = []
                    for cc in range(8):
                        xt = xp.tile([128, CH], f32r, tag="x")
                        nc.sync.dma_start(
                            xt[:], xT_d[128 * cc : 128 * cc + 128, t0 : t0 + CH]
                        )
                        xts.append(xt)
                    q_ps = qkvp.tile([128, CH], f32, tag="qkv")
                    k_ps = qkvp.tile([128, CH], f32, tag="qkv")
                    for cc in range(8):
                        nc.tensor.matmul(
                            q_ps[:], wq_sb[:, 128 * cc : 128 * cc + 128],
                            xts[cc][:], start=(cc == 0), stop=(cc == 7),
                        )
                    for cc in range(8):
                        nc.tensor.matmul(
                            k_ps[:], wk_sb[:, 128 * cc : 128 * cc + 128],
                            xts[cc][:], start=(cc == 0), stop=(cc == 7),
                        )

                    # rms statistics (rope-invariant): sq -> blockdiag-sum
                    sq_q = scr.tile([128, CH], f32r, tag="sq")
                    sq_k = scr.tile([128, CH], f32r, tag="sq")
                    nc.scalar.activation(sq_q[:], q_ps[:], AF.Square)
                    nc.scalar.activation(sq_k[:], k_ps[:], AF.Square)
                    ms_q = msp.tile([2, CH], f32, tag="ms")
                    ms_k = msp.tile([2, CH], f32, tag="ms")
                    nc.tensor.matmul(ms_q[:], bd_sb[:], sq_q[:], start=True, stop=True)
                    nc.tensor.matmul(ms_k[:], bd_sb[:], sq_k[:], start=True, stop=True)
                    def rs_chain(ms_ps, name):
                        ms_sb = scr.tile([2, CH], f32, tag="ms_" + name)
                        nc.scalar.copy(ms_sb[:], ms_ps[:])
                        m2 = scr.tile([64, 16], f32, tag="m2_" + name)
                        nc.sync.dma_start(m2[:], ms_sb[:])
                        std2 = scr.tile([64, 16], f32, tag="st_" + name)
                        nc.scalar.activation(
                            std2[:], m2[:], AF.Sqrt, scale=1.0 / D,
                            bias=epsb[0:64, :],
                        )
                        r2 = scr.tile([64, 16], f32r, tag="r2_" + name)
                        with nc.allow_low_precision(reason="rsqrt to f32r"):
                            nc.vector.reciprocal(r2[:], std2[:])
                        rs4 = scr.tile([2, CH], f32r, tag="rs_" + name)
                        nc.sync.dma_start(rs4[:], r2[:])
                        return rs4

                    rs_q4 = rs_chain(ms_q, "q")
                    rs_k4 = rs_chain(ms_k, "k")
                    rsbc_q = rsbcp.tile([128, CH], f32, tag="rsbc")
                    nc.tensor.matmul(
                        rsbc_q[:], e2_sb[:], rs_q4[:], start=True, stop=True
                    )

                    # rope: tc = x*cos (sbuf), ts = x*sin (psum, for the
                    # partition-shifted combine), o = tc -+ ts_shifted
                    tcq = scr.tile([128, 2 * CH], f32, tag="tc")
                    nc.vector.tensor_mul(
                        tcq[:, 0:CH], q_ps[:], cosq[:, tt0 : tt0 + CH]
                    )
                    nc.vector.tensor_mul(
                        tcq[:, CH : 2 * CH], k_ps[:], cosk[:, tt0 : tt0 + CH]
                    )
                    ts_q = scrp.tile([128, CH], f32, tag="tsp")
                    ts_k = scrp.tile([128, CH], f32, tag="tsp")
                    nc.vector.tensor_mul(ts_q[:], q_ps[:], sinq[:, tt0 : tt0 + CH])
                    nc.vector.tensor_mul(ts_k[:], k_ps[:], sink[:, tt0 : tt0 + CH])
                    o_ = scr.tile([128, 2 * CH], f32, tag="o_")

                    def addsub(lo, hi, sub_first):
                        # o[lo:lo+32] = tc[lo:lo+32] - ts[hi:hi+32]  (x1 rows)
                        # o[hi:hi+32] = tc[hi:hi+32] + ts[lo:lo+32]  (x2 rows)
                        for (ts_t, off) in ((ts_q, 0), (ts_k, CH)):
                            sl = slice(off, off + CH)
                            nc.vector.tensor_sub(
                                o_[lo : lo + 32, sl],
                                tcq[lo : lo + 32, sl],
                                ts_t[hi : hi + 32, :],
                            )
                            nc.vector.tensor_add(
                                o_[hi : hi + 32, sl],
                                tcq[hi : hi + 32, sl],
                                ts_t[lo : lo + 32, :],
                            )

                    addsub(0, 32, True)
                    addsub(64, 96, True)

                    nc.vector.tensor_mul(
                        qT[:, t0 : t0 + CH], o_[:, 0:CH], rsbc_q[:]
                    )
                    rsbc_k = rsbcp.tile([128, CH], f32, tag="rsbc")
                    nc.tensor.matmul(
                        rsbc_k[:], e2_sb[:], rs_k4[:], start=True, stop=True
                    )
                    nc.vector.tensor_mul(
                        kT[:, t0 : t0 + CH], o_[:, CH : 2 * CH], rsbc_k[:]
                    )

                    # v: plain projection then transpose each 128-token tile
                    v_ps = qkvp.tile([128, CH], f32, tag="qkv")
                    for cc in range(8):
                        nc.tensor.matmul(
                            v_ps[:], wv_sb[:, 128 * cc : 128 * cc + 128],
                            xts[cc][:], start=(cc == 0), stop=(cc == 7),
                        )
                    nc.scalar.copy(vT[:, t0 : t0 + CH], v_ps[:])
                    for jj in range(4):
                        kti = (tt0 // KT) + jj          # k-tile within batch
                        col = t0 + jj * KT
                        vt_ps = scrp.tile([128, 128], f32r, tag="tsp")
                        nc.tensor.transpose(
                            vt_ps[:], vT[:, col : col + 128], ident[:]
                        )
                        base = (b * NKT + kti) * 65
                        nc.scalar.copy(vaug0[:, base : base + 64], vt_ps[:, 0:64])
                        nc.scalar.copy(vaug1[:, base : base + 64], vt_ps[:, 64:128])

            # ================= phase 2: attention =================
            with (
                tc.tile_pool(name="p2sb", bufs=1) as p2,
                tc.tile_pool(name="pp2", bufs=8) as ppool,
                tc.tile_pool(name="op2", bufs=8) as opool,
                tc.tile_pool(name="ps2", bufs=3, space="PSUM") as spsp,
                tc.tile_pool(name="ps2y", bufs=1, space="PSUM") as yp,
            ):
                yraw = p2.tile([65, 4096], f32, tag="yraw")
                mask2 = p2.tile([128, 4096], f32r, tag="mask2")
                nc.sync.dma_start(mask2[:], mask2_d[:])
                den2 = p2.tile([64, 64], f32, tag="den2")
                rcp2 = p2.tile([64, 64], f32, tag="rcp2")

                for b in range(B):
                    bt = b * T
                    for qi in range(NQT):
                        q0 = bt + qi * QT
                        nk = 4 * qi + 4
                        y_ps = yp.tile([65, 2 * QT], f32, tag="y")
                        for ki in range(nk):
                            k0 = bt + ki * KT
                            s_ps = spsp.tile([128, 2 * QT], f32, tag="sps")
                            nc.tensor.matmul(
                                s_ps[:, 0:QT],
                                kT[0:64, k0 : k0 + KT],
                                qT[0:64, q0 : q0 + QT],
                                start=True, stop=True, tile_position=(0, 0),
                            )
                            nc.tensor.matmul(
                                s_ps[:, QT : 2 * QT],
                                kT[64:128, k0 : k0 + KT],
                                qT[64:128, q0 : q0 + QT],
                                start=True, stop=True, tile_position=(64, 0),
                            )
                            p_sb = ppool.tile([128, 2 * QT], f32r, tag="p")
                            nc.scalar.activation(
                                p_sb[:], s_ps[:], AF.Exp, scale=0.125
                            )
                            mi = ki - 4 * qi
                            if mi >= 0:
                                # staircase mask on the leading m+128 columns
                                msz = min(mi * KT + KT, QT)
                                pv = p_sb[:].rearrange(
                                    "p (h q) -> p h q", h=2
                                )[:, :, 0:msz]
                                mv = mask2[
                                    :, 1024 * mi : 1024 * mi + 1024
                                ].rearrange("p (h q) -> p h q", h=2)[:, :, 0:msz]
                                nc.vector.tensor_mul(pv, pv, mv)
                            base = (b * NKT + ki) * 65
                            nc.tensor.matmul(
                                y_ps[:, 0:QT],
                                vaug0[:, base : base + 65],
                                p_sb[:, 0:QT],
                                start=(ki == 0), stop=(ki == nk - 1),
                            )
                            nc.tensor.matmul(
                                y_ps[:, QT : 2 * QT],
                                vaug1[:, base : base + 65],
                                p_sb[:, QT : 2 * QT],
                                start=(ki == 0), stop=(ki == nk - 1),
                            )
                        # stash raw y + den; cols h*2048 + qi*512
                        nc.vector.tensor_copy(
                            yraw[:, 0 * T + qi * QT : 0 * T + qi * QT + QT],
                            y_ps[:, 0:QT],
                        )
                        nc.vector.tensor_copy(
                            yraw[:, 1 * T + qi * QT : 1 * T + qi * QT + QT],
                            y_ps[:, QT : 2 * QT],
                        )

                    # ---- phase 2.5: normalize + out-projection for batch b
                    nc.sync.dma_start(
                        den2[:],
                        yraw[64:65, :].rearrange("o (p f) -> o p f", p=64),
                    )
                    nc.vector.reciprocal(rcp2[:], den2[:])
                    nc.sync.dma_start(
                        yraw[64:65, :].rearrange("o (p f) -> o p f", p=64),
                        rcp2[:],
                    )
                    for h in range(2):
                        for half in range(2):
                            c0 = h * T + half * 1024
                            db_ps = spsp.tile([64, 1024], f32, tag="sps")
                            nc.tensor.matmul(
                                db_ps[:, 0:512],
                                onesr[64:65, :],
                                yraw[64:65, c0 : c0 + 512],
                                start=True, stop=True,
                            )
                            nc.tensor.matmul(
                                db_ps[:, 512:1024],
                                onesr[64:65, :],
                                yraw[64:65, c0 + 512 : c0 + 1024],
                                start=True, stop=True,
                            )
                            nc.vector.tensor_mul(
                                yT[64 * h : 64 * h + 64,
                                   bt + half * 1024 : bt + half * 1024 + 1024],
                                yraw[0:64, c0 : c0 + 1024],
                                db_ps[:],
                            )
                    for tt in range(16):
                        tg = bt + tt * 128
                        o_ps = spsp.tile([128, C], f32, tag="sps")
                        nc.tensor.matmul(
                            o_ps[:, 0:512], yT[:, tg : tg + 128],
                            wo_sb[:, 0:512], start=True, stop=True,
                        )
                        nc.tensor.matmul(
                            o_ps[:, 512:1024], yT[:, tg : tg + 128],
                            wo_sb[:, 512:1024], start=True, stop=True,
                        )
                        o_sb = opool.tile([128, C], f32, tag="osb")
                        if tt % 2 == 0:
                            nc.vector.tensor_copy(o_sb[:], o_ps[:])
                        else:
                            nc.scalar.copy(o_sb[:], o_ps[:])
                        nc.sync.dma_start(out_d[tg : tg + 128, :], o_sb[:])

    nc.finalize()
    return nc


def _host_prep(x, w_qkv, w_out, q_norm_w, k_norm_w):
    xT = np.ascontiguousarray(x.reshape(TT, C).T).astype(np.float32)

    j = np.arange(32, dtype=np.float64)
    inv = ROPE_BASE ** (-j / 32.0)
    tt = np.arange(T, dtype=np.float64)
    ang = tt[:, None] * inv[None, :]          # [T, 32]
    cos_t = np.cos(ang)                        # [T, 32]
    sin_t = np.sin(ang)

    def trig_tables(w):
        w = np.asarray(w, dtype=np.float64)
        cosr = np.empty((128, T), np.float32)
        sinr = np.empty((128, T), np.float32)
        for p in range(128):
            jj = p % 32
            r = p % 64
            cosr[p] = (cos_t[:, jj] * w[r]).astype(np.float32)
            sinr[p] = (sin_t[:, jj] * w[(r + 32) % 64]).astype(np.float32)
        return cosr, sinr

    cosq, sinq = trig_tables(q_norm_w)
    cosk, sink = trig_tables(k_norm_w)

    mask2 = np.zeros((128, 4096), np.float32)
    kp = np.arange(128)[:, None]
    qq = np.arange(512)[None, :]
    for mi, m in enumerate((0, 128, 256, 384)):
        blk = (kp + m <= qq).astype(np.float32)
        mask2[:, 1024 * mi : 1024 * mi + 512] = blk
        mask2[:, 1024 * mi + 512 : 1024 * mi + 1024] = blk

    e2 = np.zeros((2, 128), np.float32)
    e2[0, 0:64] = 1.0
    e2[1, 64:128] = 1.0
    bd = np.zeros((128, 2), np.float32)
    bd[0:64, 0] = 1.0
    bd[64:128, 1] = 1.0
    onesr = np.ones((65, 64), np.float32)
    ones32 = np.ones((128, 32), np.float32)
    ident = np.eye(128, dtype=np.float32)

    shared = {
        "xT": xT, "cosq": cosq, "sinq": sinq, "cosk": cosk, "sink": sink,
        "mask2": mask2, "e2": e2, "bd": bd, "onesr": onesr,
        "ones32": ones32, "ident": ident,
        "epsb": np.full((128, 1), EPS, np.float32),
    }

    in_maps = []
    for c in range(NC):
        rows = np.r_[2 * c * 64 : (2 * c + 1) * 64,
                     (2 * c + 1) * 64 : (2 * c + 2) * 64]
        wq = w_qkv[rows, :]
        wk = w_qkv[C + rows, :]
        wv = w_qkv[2 * C + rows, :]
        m = dict(shared)
        m["wqT"] = np.ascontiguousarray(wq.T).astype(np.float32)
        m["wkT"] = np.ascontiguousarray(wk.T).astype(np.float32)
        m["wvT"] = np.ascontiguousarray(wv.T).astype(np.float32)
        m["woT"] = np.ascontiguousarray(w_out[:, rows].T).astype(np.float32)
        in_maps.append(m)
    return in_maps


def kernel(x, w_qkv, w_out, q_norm_w, k_norm_w, _trace=False):
    from concourse.bass_utils import run_bass_kernel_spmd

    if "nc" not in _cache:
        _cache["nc"] = _build()
    nc = _cache["nc"]

    x = np.asarray(x, dtype=np.float32)
    w_qkv = np.asarray(w_qkv, dtype=np.float32)
    w_out = np.asarray(w_out, dtype=np.float32)
    q_norm_w = np.asarray(q_norm_w, dtype=np.float32)
    k_norm_w = np.asarray(k_norm_w, dtype=np.float32)

    in_maps = _host_prep(x, w_qkv, w_out, q_norm_w, k_norm_w)
    res = run_bass_kernel_spmd(
        nc, in_maps, list(range(NC)), trace=_trace,
    )
    _cache["last_result"] = res
    parts = np.stack([r["out"] for r in res.results], axis=0)
    out = parts.sum(axis=0, dtype=np.float64).astype(np.float32)
    return out.reshape(B, T, C)
